# revision 31
# baseline (speedup 1.0000x reference)
"""Trainium2 Bass kernel for nn_HallucinationDetector.

Computes, per batch sample b:
    risk[b] = clip( 0.25 * routing_entropy[b]/ln(8)
                  - 0.2  * (1 - moe_confidence[b])
                  + 0.2  * sigmoid(memory_mismatch[b] - 2)
                  + 0.2  * mean_s sigmoid(hidden[b,s,:] @ probe_w + probe_b)
                  + 0.15 * sigmoid(1/(||routing_repr[b]|| + 1e-8) - 1), 0, 1)

Sharding: pure data-parallel over the batch dim across 8 NeuronCores
(128 samples per core). Layout on each core: partition = local sample
index (exactly 128), free dim = (seq, hidden). The dominant work — the
per-token probe dot product over 1 GiB of hidden_states — is fused DVE
scalar_tensor_tensor (multiply + free-dim-reduce in one instruction),
one per sequence position, overlapped with the hidden_states stream.

DMA strategy (measured on the axon-tunneled trn2 cores): with all 8
cores streaming, per-core HBM read rate collapses from ~410 GB/s
(single core) to ~190 GB/s unless many DMA instructions are in flight.
Best found: 1 MiB tiles (s_tile=4 -> 8 KiB/partition descriptors),
alternating the two HWDGE rings (qSPDynamicHW / qActDynamicHW), with a
20-deep buffer pool (10 outstanding DMAs per ring). All small loads go
on the SWDGE (gpsimd) queue so the rings carry only hidden_states.
"""

import math
from contextlib import ExitStack

import numpy as np

import concourse.bass as bass
import concourse.bacc as bacc
import concourse.tile as tile
from concourse import mybir
from concourse.bass_utils import run_bass_kernel_spmd

# Problem shapes (hardcoded; kernel.py must be self-contained).
B, S, H, D = 1024, 512, 512, 2048
N_CORES = 8
BPC = B // N_CORES  # 128 samples per core == SBUF partition count

MAX_ENTROPY = math.log(8.0)
W_ENTROPY, W_CONF, W_MISMATCH, W_SEMANTIC, W_EIGEN = 0.25, -0.2, 0.2, 0.2, 0.15

F32 = mybir.dt.float32
BF16 = mybir.dt.bfloat16

# hidden_states is cast to bf16 on the host before upload: halves the
# HBM traffic of the dominant stream (the probe dot product accumulates
# in fp32 inside the DVE pipeline; measured end-to-end rel err ~1e-4).
HID_BF16 = True


def build_nc(
    s_tile: int = 8,
    hid_bufs: int = 20,
    dma_queues: int = 2,
    gp_every: int = 0,
    repeats: int = 1,
    no_compute: bool = False,
    no_dma: bool = False,
    hid_bf16: bool = HID_BF16,
    act_per_tile: int = 4,
    gp_per_tile: int = 0,
    prod_bufs: int = 6,
):
    """Build the per-core Bass program. Identical on every core (pure SPMD,
    no collectives); each core sees its own 128-sample shard.

    repeats > 1 re-runs the whole body N times (same result) — a timing
    device: HW time/iter = (wall(R2) - wall(R1)) / (R2 - R1).
    gp_every=k routes every k-th hidden tile over the SWDGE queue."""
    nc = bacc.Bacc("TRN2", target_bir_lowering=False, debug=True)

    dt_hid = BF16 if hid_bf16 else F32
    hid = nc.dram_tensor("hidden_states", [BPC, S, H], dt_hid, kind="ExternalInput")
    rr = nc.dram_tensor("routing_repr", [BPC, D], F32, kind="ExternalInput")
    re = nc.dram_tensor("routing_entropy", [BPC], F32, kind="ExternalInput")
    mc = nc.dram_tensor("moe_confidence", [BPC], F32, kind="ExternalInput")
    mm = nc.dram_tensor("memory_mismatch", [BPC], F32, kind="ExternalInput")
    pw = nc.dram_tensor("probe_w", [H], F32, kind="ExternalInput")
    pb = nc.dram_tensor("probe_b", [1], F32, kind="ExternalInput")
    risk = nc.dram_tensor("risk", [BPC], F32, kind="ExternalOutput")

    mult, add = mybir.AluOpType.mult, mybir.AluOpType.add
    HD = D // 2  # routing_repr processed in two chunks to save SBUF

    if not hid_bf16:
        act_per_tile = 0  # 2x-mode TT needs a 2-byte dtype

    with ExitStack() as ctx:
        tc = ctx.enter_context(tile.TileContext(nc))
        singles = ctx.enter_context(tc.tile_pool(name="singles", bufs=1))
        hid_pool = ctx.enter_context(tc.tile_pool(name="hid", bufs=hid_bufs))
        prod_pool = ctx.enter_context(tc.tile_pool(name="prod", bufs=prod_bufs))

      # fmt: off
        def body():
            # --- replicated constants (SWDGE: step-0 partition APs; the DMA
            # also casts f32 -> bf16 when dt_hid is bf16) --------------------
            w_tile = singles.tile([BPC, H], dt_hid)
            w_src = pw[:]
            w_bcast = bass.AP(
                tensor=w_src.tensor, offset=w_src.offset, ap=[[0, BPC]] + list(w_src.ap)
            )
            nc.gpsimd.dma_start(out=w_tile, in_=w_bcast)

            b_tile = singles.tile([BPC, 1], F32)
            b_src = pb[:]
            b_bcast = bass.AP(
                tensor=b_src.tensor, offset=b_src.offset, ap=[[0, BPC]] + list(b_src.ap)
            )
            nc.gpsimd.dma_start(out=b_tile, in_=b_bcast)

            # --- small per-sample vectors: [128] dram -> [128, 1] sbuf ---------
            def load_col(src, nm):
                t = singles.tile([BPC, 1], F32, tag=nm, name=nm)
                nc.gpsimd.dma_start(out=t, in_=src[:].rearrange("(p o) -> p o", o=1))
                return t

            re_t = load_col(re, "re_t")
            mc_t = load_col(mc, "mc_t")
            mm_t = load_col(mm, "mm_t")

            # --- eigen-branch data loads (compute deferred to the tail so no
            # ACT-engine work sits ahead of ring-B's DMA issue) ----------------
            rr_ts = []
            for ci in range(2):
                rr_t = singles.tile([BPC, HD], F32, tag=f"rr_t{ci}", name=f"rr_t{ci}")
                nc.gpsimd.dma_start(out=rr_t, in_=rr[:, ci * HD : (ci + 1) * HD])
                rr_ts.append(rr_t)

            # --- main loop: logits[b, s] = hidden[b, s, :] @ probe_w -----------
            logits = singles.tile([BPC, S], F32, tag="logits")
            scr = singles.tile([BPC, H], dt_hid, tag="scr")  # STT's (unused) out
            ascr = singles.tile([BPC, H], BF16, tag="ascr")  # ACT's (unused) out
            gscr = None
            if gp_per_tile:
                gscr = singles.tile([BPC, H], BF16, tag="gscr")  # GP's out
            if no_compute:
                nc.vector.memset(logits, 0.0)
            static_ht = None
            if no_dma:
                static_ht = hid_pool.tile([BPC, s_tile, H], dt_hid, tag="static_ht")
                nc.vector.memset(static_ht, 0.01)
            def eigen_block():
                # eigen-score branch: ss = ||rr||^2 in two chunks. Emitted
                # mid-loop (pipeline deep) so its ACT ops + activation-table
                # loads stall neither ring-B's first DMA issues (head) nor
                # the epilogue chain (tail).
                rr_scr = singles.tile([BPC, HD], F32, tag="rr_scr")
                ss_parts = []
                for ci in range(2):
                    ssp = singles.tile([BPC, 1], F32, tag=f"ss{ci}", name=f"ss{ci}")
                    # fused square + free-dim-sum: out = (rr*1)*rr, accum = sum
                    nc.vector.scalar_tensor_tensor(
                        out=rr_scr, in0=rr_ts[ci], scalar=1.0, in1=rr_ts[ci],
                        op0=mult, op1=mult, accum_out=ssp,
                    )
                    ss_parts.append(ssp)
                ss = singles.tile([BPC, 1], F32, tag="ss")
                nc.vector.tensor_tensor(out=ss, in0=ss_parts[0], in1=ss_parts[1], op=add)
                sv = singles.tile([BPC, 1], F32, tag="sv")
                nc.scalar.sqrt(sv, ss)
                nc.vector.tensor_scalar_add(sv, sv, 1e-8)
                eig = singles.tile([BPC, 1], F32, tag="eig")
                nc.vector.reciprocal(eig, sv)
                neg1 = singles.tile([BPC, 1], F32, tag="neg1")
                nc.vector.memset(neg1, -1.0)
                neg2 = singles.tile([BPC, 1], F32, tag="neg2")
                nc.vector.memset(neg2, -2.0)
                ne2 = singles.tile([BPC, 1], F32, tag="ne2")
                nc.scalar.activation(
                    ne2, eig, mybir.ActivationFunctionType.Sigmoid, bias=neg1, scale=1.0
                )
                nm = singles.tile([BPC, 1], F32, tag="nm")
                nc.scalar.activation(
                    nm, mm_t, mybir.ActivationFunctionType.Sigmoid, bias=neg2, scale=1.0
                )
                return ne2, nm

            eigen_res = None
            rings = [nc.sync, nc.scalar][: max(1, dma_queues)]
            ring_i = 0
            for i in range(S // s_tile):
                if no_dma:
                    ht = static_ht
                else:
                    ht = hid_pool.tile([BPC, s_tile, H], dt_hid)
                    if gp_every and (i % gp_every == gp_every - 1):
                        q = nc.gpsimd
                    else:
                        q = rings[ring_i % len(rings)]
                        ring_i += 1
                    q.dma_start(
                        out=ht, in_=hid[:, i * s_tile : (i + 1) * s_tile, :]
                    )
                if no_compute:
                    continue
                # DVE+ACT split: the first act_per_tile tokens go through one
                # batched 2x-mode DVE tensor_tensor multiply (bf16 packed)
                # whose per-token free-dim reductions run on the otherwise-
                # idle ACT engine (Identity activation with accumulator
                # read); the rest use the fused 1x DVE scalar_tensor_tensor.
                # Balances the two engines under the DMA stream time.
                if act_per_tile:
                    prod_t = prod_pool.tile([BPC, act_per_tile, H], BF16, name="prod_t")
                    w_rep = bass.AP(
                        tensor=w_tile.tensor, offset=w_tile.offset,
                        ap=[list(w_tile.ap[0]), [0, act_per_tile], list(w_tile.ap[1])],
                    )
                    nc.vector.tensor_tensor(
                        out=prod_t, in0=ht[:, :act_per_tile, :], in1=w_rep, op=mult
                    )
                    for j in range(act_per_tile):
                        s_idx = i * s_tile + j
                        nc.scalar.activation(
                            ascr, prod_t[:, j, :],
                            mybir.ActivationFunctionType.Identity,
                            accum_out=logits[:, s_idx : s_idx + 1],
                        )
                for j in range(act_per_tile, act_per_tile + gp_per_tile):
                    s_idx = i * s_tile + j
                    # third engine: fused dot product on GpSimd (Pool)
                    nc.gpsimd.scalar_tensor_tensor(
                        out=gscr, in0=ht[:, j, :], scalar=1.0, in1=w_tile,
                        op0=mult, op1=mult, accum_out=logits[:, s_idx : s_idx + 1],
                    )
                for j in range(act_per_tile + gp_per_tile, s_tile):
                    s_idx = i * s_tile + j
                    # fused dot product: out = (h*1.0)*w, accum_out = sum(out)
                    nc.vector.scalar_tensor_tensor(
                        out=scr, in0=ht[:, j, :], scalar=1.0, in1=w_tile,
                        op0=mult, op1=mult, accum_out=logits[:, s_idx : s_idx + 1],
                    )
                if i == 16:
                    eigen_res = eigen_block()
            if eigen_res is None:
                eigen_res = eigen_block()

            ne2, nm = eigen_res
            # --- semantic entropy: mean_s sigmoid(logits + b) ------------------
            probs = singles.tile([BPC, S], F32, tag="scr", name="probs")  # reuse
            nc.scalar.activation(
                probs, logits, mybir.ActivationFunctionType.Sigmoid, bias=b_tile, scale=1.0
            )
            sum_p = singles.tile([BPC, 1], F32, tag="sum_p")
            nc.vector.tensor_reduce(sum_p, probs, mybir.AxisListType.X, add)

            # --- weighted fusion + clip ----------------------------------------
            # risk = (-0.2) + (W_E/ln8)*re + 0.2*mc + 0.2*nm + (0.2/512)*sum_p + 0.15*ne2
            a0 = singles.tile([BPC, 1], F32, tag="a0")
            nc.vector.tensor_scalar(
                out=a0, in0=re_t, scalar1=W_ENTROPY / MAX_ENTROPY, scalar2=W_CONF,
                op0=mult, op1=add,
            )
            _fma_n = [0]

            def fma(x, c, acc):
                _fma_n[0] += 1
                o = singles.tile([BPC, 1], F32, tag=f"fma{_fma_n[0]}", name="fma")
                nc.vector.scalar_tensor_tensor(
                    out=o, in0=x, scalar=c, in1=acc, op0=mult, op1=add
                )
                return o

            a1 = fma(mc_t, -W_CONF, a0)
            a2 = fma(nm, W_MISMATCH, a1)
            a3 = fma(sum_p, W_SEMANTIC / S, a2)
            a4 = fma(ne2, W_EIGEN, a3)
            out_t = singles.tile([BPC, 1], F32, tag="out_t")
            nc.vector.tensor_scalar(
                out=out_t, in0=a4, scalar1=0.0, scalar2=1.0,
                op0=mybir.AluOpType.max, op1=mybir.AluOpType.min,
            )
            nc.sync.dma_start(
                out=risk[:].rearrange("(p o) -> p o", o=1), in_=out_t
            )

        for _rep in range(repeats):
            body()

    nc.finalize()
    return nc


_NC_CACHE: dict = {}


def _get_nc(**kw):
    key = tuple(sorted(kw.items()))
    if key not in _NC_CACHE:
        _NC_CACHE[key] = build_nc(**kw)
    return _NC_CACHE[key]


def _make_in_maps(inputs: dict, stagger: int = 0, hid_bf16: bool = HID_BF16) -> list:
    hs = np.ascontiguousarray(np.asarray(inputs["hidden_states"], dtype=np.float32))
    if hid_bf16:
        hs = hs.astype(mybir.dt.np(BF16))  # round-to-nearest-even
    rr = np.ascontiguousarray(np.asarray(inputs["routing_repr"], dtype=np.float32))
    re = np.asarray(inputs["routing_entropy"], dtype=np.float32)
    mc = np.asarray(inputs["moe_confidence"], dtype=np.float32)
    mm = np.asarray(inputs["memory_mismatch"], dtype=np.float32)
    pw = np.asarray(inputs["probe_w"], dtype=np.float32)
    pb = np.asarray(inputs["probe_b"], dtype=np.float32)
    maps = []
    for c in range(N_CORES):
        sl = slice(c * BPC, (c + 1) * BPC)
        hs_c = hs[sl]
        if stagger:
            # mean over tokens is permutation-invariant: rotating each
            # core's token axis de-phases the 8 cores' HBM read streams
            hs_c = np.ascontiguousarray(np.roll(hs_c, c * stagger, axis=1))
        maps.append(
            {
                "hidden_states": hs_c,
                "routing_repr": rr[sl],
                "routing_entropy": re[sl],
                "moe_confidence": mc[sl],
                "memory_mismatch": mm[sl],
                "probe_w": pw,
                "probe_b": pb,
            }
        )
    return maps


def run(inputs: dict, trace: bool = False, stagger: int = 0, **build_kw):
    """Run the kernel on 8 cores; returns (risk[1024] f32, BassKernelResults)."""
    nc = _get_nc(**build_kw)
    in_maps = _make_in_maps(
        inputs, stagger=stagger, hid_bf16=build_kw.get("hid_bf16", HID_BF16)
    )
    res = run_bass_kernel_spmd(nc, in_maps, list(range(N_CORES)), trace=trace)
    out = np.concatenate([res.results[c]["risk"] for c in range(N_CORES)])
    return out.astype(np.float32, copy=False), res


def kernel(**inputs) -> np.ndarray:
    out, _ = run(inputs)
    return out
